# revision 6
# baseline (speedup 1.0000x reference)
"""Trainium2 Bass kernel for quantized BertOutput (BiT SymQuantizer 8-bit
linear + residual + LayerNorm), data-parallel over 8 NeuronCores.

Contract: kernel(**inputs) takes the FULL inputs from setup_inputs() and
returns the FULL [4, 4096, 1024] fp32 output.

Math per core (token shard of 2048 rows):
  k_x = clip(round_half_even(x * s_x), -127, 127)   (integers, bf16-exact)
  k_w = round_half_even(w * s_w)
  h   = (k_x @ k_w.T) * inv_ss                       (bf16 matmul, fp32 PSUM)
  y   = h + res ; out = (y - mean(y)) * rsqrt(var(y) + eps)

The BiT layerwise scales need global abs-maxes *before* any quantized tile
can be produced, so they are obtained in a tiny first launch (k1): each core
reduces one 128-row slice of W on-device; the host only max-combines the 8
device-computed scalars and forms s_w = 127/min(m,2.5) in fp32 (bit-identical
to the reference's divide).  s_x = 127/2.5 is *assumed* (the clip saturates
for any realistic input) and *proven* on device: k2 reports an abs-max over
its first row-tile, and if no core saw an element >= 2.5 the host falls back
to a general-scale kernel, so the result is correct for arbitrary inputs.
"""

from contextlib import ExitStack

import numpy as np

import concourse.bacc as bacc
import concourse.bass as bass
import concourse.mybir as mybir
from concourse import bass_isa  # noqa: F401
from concourse.bass_utils import run_bass_kernel_spmd
from concourse.tile import TileContext

F32 = mybir.dt.float32
BF16 = mybir.dt.bfloat16
MAGIC = float(np.float32(12582912.0))  # 1.5 * 2**23 -> fp32 RNE round trick
AX = mybir.AxisListType.X
ALU = mybir.AluOpType

B, S, INTER, HID = 4, 4096, 4096, 1024
N_CORES = 8
TOK = (B * S) // N_CORES  # 2048 tokens per core
CLIP = 2.5
EPS = 1e-12

_NC_CACHE: dict = {}
LAST_EXEC_NS: list = []  # (label, exec_time_ns) when BERT_KERNEL_TRACE=1


def _build_absmax(rows: int, cols: int):
    """out[0,0] = max(|inp|) over [rows, cols]."""
    nc = bacc.Bacc("TRN2", target_bir_lowering=False, debug=False)
    inp = nc.declare_dram_parameter("inp", [rows, cols], F32, isOutput=False)
    outp = nc.declare_dram_parameter("absmax", [1, 1], F32, isOutput=True)
    scr = nc.dram_tensor("scr", [128, 1], F32)
    n_chunks = rows // 128
    with TileContext(nc) as tc:
        with tc.tile_pool(name="pool", bufs=2) as pool, tc.tile_pool(
            name="small", bufs=1
        ) as small:
            cols_t = small.tile([128, max(n_chunks, 2)], F32)
            for c in range(n_chunks):
                t = pool.tile([128, cols], F32)
                nc.sync.dma_start(out=t[:], in_=inp[c * 128 : (c + 1) * 128, :])
                nc.vector.tensor_reduce(
                    out=cols_t[:, c : c + 1], in_=t[:], axis=AX,
                    op=ALU.max, apply_absolute_value=True,
                )
            m = small.tile([128, 1], F32)
            nc.vector.tensor_reduce(
                out=m[:], in_=cols_t[:, 0:n_chunks], axis=AX, op=ALU.max
            )
            nc.sync.dma_start(out=scr[:], in_=m[:])
            row = small.tile([1, 128], F32)
            nc.sync.dma_start(out=row[:], in_=scr[:].rearrange("p one -> one p"))
            mall = small.tile([1, 1], F32)
            nc.vector.tensor_reduce(out=mall[:], in_=row[:], axis=AX, op=ALU.max)
            nc.sync.dma_start(out=outp[:], in_=mall[:])
    nc.compile()
    return nc


def _build_main(
    general_affine: bool,
    clamp_w: bool,
    s_x_const: float,
    TOKc: int = TOK,
    K: int = INTER,
    HIDc: int = HID,
):
    TOK_T = TOKc // 128
    KT = K // 128
    KSW = 512
    KS = K // KSW
    HID_T = HIDc // 128
    XH = min(K, 2048)
    XHN = K // XH
    NB = min(512, HIDc)

    nc = bacc.Bacc("TRN2", target_bir_lowering=False, debug=False)
    x_h = nc.declare_dram_parameter("x", [TOKc, K], F32, isOutput=False)
    res_h = nc.declare_dram_parameter("res", [TOKc, HIDc], F32, isOutput=False)
    w_h = nc.declare_dram_parameter("W", [HIDc, K], F32, isOutput=False)
    scal_h = nc.declare_dram_parameter("scal", [1, 2], F32, isOutput=False)
    aff_h = nc.declare_dram_parameter("aff", [3, HIDc], F32, isOutput=False)
    out_h = nc.declare_dram_parameter("out", [TOKc, HIDc], F32, isOutput=True)
    stat_h = nc.declare_dram_parameter("stats", [1, 1], F32, isOutput=True)
    scr = nc.dram_tensor("scr", [128, 1], F32)

    w_r = w_h[:].rearrange("(g p) k -> p g k", p=128)

    with TileContext(nc) as tc, ExitStack() as ctx:
        small = ctx.enter_context(tc.tile_pool(name="small", bufs=1))
        wload = ctx.enter_context(tc.tile_pool(name="wload", bufs=2))
        wqp = ctx.enter_context(tc.tile_pool(name="wq", bufs=2))
        wqtp = ctx.enter_context(tc.tile_pool(name="wqt", bufs=1))
        xrow = ctx.enter_context(tc.tile_pool(name="xrow", bufs=3))
        xqp = ctx.enter_context(tc.tile_pool(name="xq", bufs=2))
        xqtp = ctx.enter_context(tc.tile_pool(name="xqt", bufs=3))
        resp = ctx.enter_context(tc.tile_pool(name="res", bufs=2))
        yp = ctx.enter_context(tc.tile_pool(name="y", bufs=2))
        bnp = ctx.enter_context(tc.tile_pool(name="bn", bufs=2))
        tiny = ctx.enter_context(tc.tile_pool(name="tiny", bufs=TOK_T + 2))
        psum = ctx.enter_context(tc.tile_pool(name="psum", bufs=3, space="PSUM"))

        scb = small.tile([128, 2], F32)
        nc.gpsimd.dma_start(out=scb[:], in_=scal_h[:].broadcast_to([128, 2]))
        s_w_ap = scb[:, 0:1]
        inv_ss_ap = scb[:, 1:2]

        if general_affine:
            b_rep = small.tile([128, HIDc], F32)
            g_rep = small.tile([128, HIDc], F32)
            be_rep = small.tile([128, HIDc], F32)
            nc.gpsimd.dma_start(
                out=b_rep[:], in_=aff_h[0:1, :].broadcast_to([128, HIDc]))
            nc.gpsimd.dma_start(
                out=g_rep[:], in_=aff_h[1:2, :].broadcast_to([128, HIDc]))
            nc.gpsimd.dma_start(
                out=be_rep[:], in_=aff_h[2:3, :].broadcast_to([128, HIDc]))

        # --- W stream: quantize k-slices and DMA-transpose into wqT[k, hid]
        wqT = wqtp.tile([128, KT, HIDc], BF16)
        for s in range(KS):
            wsl = wload.tile([128, HID_T, KSW], F32)
            nc.sync.dma_start(out=wsl[:], in_=w_r[:, :, s * KSW : (s + 1) * KSW])
            flat = wsl[:].rearrange("p g k -> p (g k)")
            if clamp_w:
                nc.vector.tensor_scalar(
                    out=flat, in0=flat, scalar1=-CLIP, scalar2=CLIP,
                    op0=ALU.max, op1=ALU.min,
                )
            nc.vector.tensor_scalar(
                out=flat, in0=flat, scalar1=s_w_ap, scalar2=MAGIC,
                op0=ALU.mult, op1=ALU.add,
            )
            wq = wqp.tile([128, HID_T, KSW], BF16)
            nc.vector.tensor_scalar(
                out=wq[:].rearrange("p g k -> p (g k)"), in0=flat,
                scalar1=MAGIC, scalar2=None, op0=ALU.subtract,
            )
            ktq = KSW // 128
            for g in range(HID_T):
                nc.scalar.dma_start_transpose(
                    out=wqT[:, s * ktq : (s + 1) * ktq, g * 128 : (g + 1) * 128],
                    in_=wq[:, g, :],
                )

        # --- x rows: quantize + transpose, then matmul + residual + LN
        x0m = small.tile([128, 1], F32)
        for tt in range(TOK_T):
            xqt = xqtp.tile([128, KT, 128], BF16)
            for h in range(XHN):
                xr = xrow.tile([128, XH], F32)
                nc.sync.dma_start(
                    out=xr[:],
                    in_=x_h[tt * 128 : (tt + 1) * 128, h * XH : (h + 1) * XH],
                )
                if tt == 0 and h == 0:
                    nc.vector.tensor_reduce(
                        out=x0m[:], in_=xr[:], axis=AX, op=ALU.max,
                        apply_absolute_value=True,
                    )
                nc.vector.tensor_scalar(
                    out=xr[:], in0=xr[:], scalar1=s_x_const, scalar2=MAGIC,
                    op0=ALU.mult, op1=ALU.add,
                )
                nc.vector.tensor_scalar(
                    out=xr[:], in0=xr[:], scalar1=MAGIC, scalar2=-127.0,
                    op0=ALU.subtract, op1=ALU.max,
                )
                xq = xqp.tile([128, XH], BF16)
                nc.vector.tensor_scalar(
                    out=xq[:], in0=xr[:], scalar1=127.0, scalar2=None, op0=ALU.min
                )
                kth = XH // 128
                nc.scalar.dma_start_transpose(
                    out=xqt[:, h * kth : (h + 1) * kth, :], in_=xq[:]
                )

            rt = resp.tile([128, HIDc], F32)
            nc.sync.dma_start(out=rt[:], in_=res_h[tt * 128 : (tt + 1) * 128, :])

            pt = psum.tile([128, HIDc], F32)
            for kt in range(KT):
                for n0 in range(0, HIDc, NB):
                    nc.tensor.matmul(
                        pt[:, n0 : n0 + NB],
                        xqt[:, kt, :],
                        wqT[:, kt, n0 : n0 + NB],
                        start=(kt == 0),
                        stop=(kt == KT - 1),
                    )

            y = yp.tile([128, HIDc], F32)
            nc.vector.tensor_scalar(
                out=y[:], in0=pt[:], scalar1=inv_ss_ap, scalar2=None, op0=ALU.mult
            )
            nc.vector.tensor_tensor(out=y[:], in0=y[:], in1=rt[:], op=ALU.add)
            if general_affine:
                nc.vector.tensor_tensor(out=y[:], in0=y[:], in1=b_rep[:], op=ALU.add)

            chunk = min(512, HIDc)
            nb = HIDc // chunk
            st6 = bnp.tile([128, nb * 6], F32)
            for i in range(nb):
                nc.vector.bn_stats(
                    out=st6[:, 6 * i : 6 * i + 6],
                    in_=y[:, i * chunk : (i + 1) * chunk],
                )
            mv = bnp.tile([128, 2], F32)
            nc.vector.bn_aggr(out=mv[:], in_=st6[:])

            t4 = tiny.tile([128, 6], F32)
            z = t4[:, 0:1]
            nc.vector.tensor_scalar(
                out=z, in0=mv[:, 1:2], scalar1=EPS, scalar2=None, op0=ALU.add
            )
            s0 = t4[:, 1:2]
            nc.scalar.activation(out=s0, in_=z, func=mybir.ActivationFunctionType.Sqrt)
            r0 = t4[:, 2:3]
            nc.vector.reciprocal(out=r0, in_=s0)
            q1 = t4[:, 3:4]
            nc.vector.tensor_tensor(out=q1, in0=r0, in1=r0, op=ALU.mult)
            nc.vector.tensor_tensor(out=q1, in0=q1, in1=z, op=ALU.mult)
            nc.vector.tensor_scalar(
                out=q1, in0=q1, scalar1=-0.5, scalar2=1.5, op0=ALU.mult, op1=ALU.add
            )
            r1 = t4[:, 4:5]
            nc.vector.tensor_tensor(out=r1, in0=r0, in1=q1, op=ALU.mult)
            nc.vector.tensor_scalar(
                out=y[:], in0=y[:], scalar1=mv[:, 0:1], scalar2=r1,
                op0=ALU.subtract, op1=ALU.mult,
            )
            if general_affine:
                nc.vector.tensor_tensor(out=y[:], in0=y[:], in1=g_rep[:], op=ALU.mult)
                nc.vector.tensor_tensor(out=y[:], in0=y[:], in1=be_rep[:], op=ALU.add)
            nc.sync.dma_start(out=out_h[tt * 128 : (tt + 1) * 128, :], in_=y[:])

        nc.sync.dma_start(out=scr[:], in_=x0m[:])
        xrowm = small.tile([1, 128], F32)
        nc.sync.dma_start(out=xrowm[:], in_=scr[:].rearrange("p one -> one p"))
        xga = small.tile([1, 1], F32)
        nc.vector.tensor_reduce(out=xga[:], in_=xrowm[:], axis=AX, op=ALU.max)
        nc.sync.dma_start(out=stat_h[:], in_=xga[:])
    nc.compile()
    return nc


def _get_nc(key, builder, *args):
    if key not in _NC_CACHE:
        _NC_CACHE[key] = builder(*args)
    return _NC_CACHE[key]


def _install_ntff_shim():
    """This image lacks ``antenv.axon_hooks``; synthesize it so
    run_bass_kernel_spmd(trace=True) can drive NTFF profiling through
    libaxon_pjrt.so's C ABI (same mechanism as trn_boot's ctypes hook)."""
    import contextlib
    import ctypes
    import sys
    import types

    if "antenv.axon_hooks" in sys.modules:
        return
    so_path = "/opt/axon/libaxon_pjrt.so"
    lib = ctypes.CDLL(so_path)
    if not hasattr(lib, "axon_start_nrt_profile"):
        return
    lib.axon_start_nrt_profile.argtypes = [
        ctypes.POINTER(ctypes.c_int64), ctypes.c_size_t,
    ]
    lib.axon_start_nrt_profile.restype = ctypes.c_int64
    lib.axon_stop_nrt_profile.argtypes = [ctypes.c_char_p]
    lib.axon_stop_nrt_profile.restype = ctypes.c_int64

    @contextlib.contextmanager
    def _hook(output_dir, device_ids):
        import jax

        jax.devices()
        if device_ids:
            ids = (ctypes.c_int64 * len(device_ids))(*device_ids)
            rc = lib.axon_start_nrt_profile(ids, len(device_ids))
        else:
            rc = lib.axon_start_nrt_profile(None, 0)
        if rc != 0:
            raise RuntimeError(f"axon_start_nrt_profile rc={rc}")
        try:
            yield
        finally:
            n = lib.axon_stop_nrt_profile(str(output_dir).encode())
            print(f"ntff profile: {n} file(s) -> {output_dir}", file=sys.stderr)

    mod = types.ModuleType("antenv.axon_hooks")
    mod.get_axon_ntff_profile_hook = lambda: _hook
    mod.set_axon_ntff_profile_hook = lambda h: None
    pkg = sys.modules.get("antenv") or types.ModuleType("antenv")
    pkg.axon_hooks = mod
    sys.modules["antenv"] = pkg
    sys.modules["antenv.axon_hooks"] = mod


def _run(nc, in_maps, label):
    import os

    trace = bool(os.environ.get("BERT_KERNEL_TRACE"))
    core_ids = list(range(len(in_maps)))
    if trace:
        try:
            _install_ntff_shim()
            r = run_bass_kernel_spmd(nc, in_maps, core_ids, trace=True)
            LAST_EXEC_NS.append((label, r.exec_time_ns))
            LAST_RESULTS[label] = r
            return r.results
        except Exception as e:  # trace plumbing must never break correctness
            print(f"trace failed ({label}): {type(e).__name__}: {e}")
    r = run_bass_kernel_spmd(nc, in_maps, core_ids, trace=False)
    return r.results


LAST_RESULTS: dict = {}


def kernel(hidden_states, input_tensor, W, b, gamma, beta):
    f32 = np.float32
    x = np.ascontiguousarray(hidden_states, dtype=f32).reshape(B * S, INTER)
    res = np.ascontiguousarray(input_tensor, dtype=f32).reshape(B * S, HID)
    Wc = np.ascontiguousarray(W, dtype=f32)
    b = np.asarray(b, f32).reshape(HID)
    gamma = np.asarray(gamma, f32).reshape(HID)
    beta = np.asarray(beta, f32).reshape(HID)

    general_affine = not (
        np.all(b == 0.0) and np.all(gamma == 1.0) and np.all(beta == 0.0)
    )
    aff = np.stack([b, gamma, beta]).astype(f32)

    # --- k1: W abs-max, one 128-row slice per core, host max-combines
    nc1 = _get_nc(("absmax", 128, INTER), _build_absmax, 128, INTER)
    slices = [Wc[i * 128 : (i + 1) * 128, :] for i in range(N_CORES)]
    r1 = _run(nc1, [{"inp": s} for s in slices], "k1_wmax")
    m_w = f32(max(f32(r["absmax"][0, 0]) for r in r1))

    m_eff = min(m_w, f32(CLIP))
    s_w = f32(127.0) / f32(m_eff)
    inv_ss = (f32(m_eff) / f32(127.0)) * (f32(CLIP) / f32(127.0))
    clamp_w = bool(m_w > CLIP)
    s_x = float(f32(127.0) / f32(CLIP))

    def run_main(s_x_const, inv_ss_val):
        nc2 = _get_nc(
            ("main", general_affine, clamp_w, float(s_x_const)),
            _build_main, general_affine, clamp_w, float(s_x_const),
        )
        scal = np.array([[s_w, inv_ss_val]], f32)
        in_maps = [
            {
                "x": x[i * TOK : (i + 1) * TOK],
                "res": res[i * TOK : (i + 1) * TOK],
                "W": Wc,
                "scal": scal,
                "aff": aff,
            }
            for i in range(N_CORES)
        ]
        r2 = _run(nc2, in_maps, "k2_main")
        out = np.concatenate([r["out"] for r in r2], axis=0)
        xmax = max(float(r["stats"][0, 0]) for r in r2)
        return out, xmax

    out, xmax = run_main(s_x, inv_ss)
    if xmax < CLIP:
        # clip never saturated in the sampled tiles: prove/refute s_x=127/2.5
        # with a full device abs-max over x, and recompute if refuted.
        ncx = _get_nc(("absmax", TOK, INTER), _build_absmax, TOK, INTER)
        rx = _run(ncx, [{"inp": x[i * TOK : (i + 1) * TOK]} for i in range(N_CORES)],
                  "kx_xmax")
        gmax = f32(max(f32(r["absmax"][0, 0]) for r in rx))
        if gmax < CLIP:
            m_x = f32(min(gmax, f32(CLIP)))
            s_x2 = f32(127.0) / m_x
            inv2 = (f32(m_x) / f32(127.0)) * (f32(m_eff) / f32(127.0))
            out, _ = run_main(float(s_x2), inv2)

    return out.reshape(B, S, HID).astype(np.float32)


# revision 7
# speedup vs baseline: 1.1459x; 1.1459x over previous
"""Trainium2 Bass kernel for quantized BertOutput (BiT SymQuantizer 8-bit
linear + residual + LayerNorm), data-parallel over 8 NeuronCores.

Contract: kernel(**inputs) takes the FULL inputs from setup_inputs() and
returns the FULL [4, 4096, 1024] fp32 output.

Math per core (token shard of 2048 rows):
  k_x = clip(round_half_even(x * s_x), -127, 127)   (integers, bf16-exact)
  k_w = round_half_even(w * s_w)
  h   = (k_x @ k_w.T) * inv_ss                       (bf16 matmul, fp32 PSUM)
  y   = h + res ; out = (y - mean(y)) * rsqrt(var(y) + eps)

The BiT layerwise scales need global abs-maxes *before* any quantized tile
can be produced, so they are obtained in a tiny first launch (k1): each core
reduces one 128-row slice of W on-device; the host only max-combines the 8
device-computed scalars and forms s_w = 127/min(m,2.5) in fp32 (bit-identical
to the reference's divide).  s_x = 127/2.5 is *assumed* (the clip saturates
for any realistic input) and *proven* on device: k2 reports an abs-max over
its first row-tile, and if no core saw an element >= 2.5 the host falls back
to a general-scale kernel, so the result is correct for arbitrary inputs.
"""

from contextlib import ExitStack

import numpy as np

import concourse.bacc as bacc
import concourse.bass as bass
import concourse.mybir as mybir
from concourse import bass_isa  # noqa: F401
from concourse.bass_utils import run_bass_kernel_spmd
from concourse.tile import TileContext

F32 = mybir.dt.float32
BF16 = mybir.dt.bfloat16
MAGIC = float(np.float32(12582912.0))  # 1.5 * 2**23 -> fp32 RNE round trick
AX = mybir.AxisListType.X
ALU = mybir.AluOpType

B, S, INTER, HID = 4, 4096, 4096, 1024
N_CORES = 8
TOK = (B * S) // N_CORES  # 2048 tokens per core
CLIP = 2.5
EPS = 1e-12

_NC_CACHE: dict = {}
LAST_EXEC_NS: list = []  # (label, exec_time_ns) when BERT_KERNEL_TRACE=1


def _build_absmax(rows: int, cols: int):
    """out[0,0] = max(|inp|) over [rows, cols]."""
    nc = bacc.Bacc("TRN2", target_bir_lowering=False, debug=False)
    inp = nc.declare_dram_parameter("inp", [rows, cols], F32, isOutput=False)
    outp = nc.declare_dram_parameter("absmax", [1, 1], F32, isOutput=True)
    scr = nc.dram_tensor("scr", [128, 1], F32)
    n_chunks = rows // 128
    with TileContext(nc) as tc:
        with tc.tile_pool(name="pool", bufs=2) as pool, tc.tile_pool(
            name="small", bufs=1
        ) as small:
            cols_t = small.tile([128, max(n_chunks, 2)], F32)
            for c in range(n_chunks):
                t = pool.tile([128, cols], F32)
                nc.sync.dma_start(out=t[:], in_=inp[c * 128 : (c + 1) * 128, :])
                nc.vector.tensor_reduce(
                    out=cols_t[:, c : c + 1], in_=t[:], axis=AX,
                    op=ALU.max, apply_absolute_value=True,
                )
            m = small.tile([128, 1], F32)
            nc.vector.tensor_reduce(
                out=m[:], in_=cols_t[:, 0:n_chunks], axis=AX, op=ALU.max
            )
            nc.sync.dma_start(out=scr[:], in_=m[:])
            row = small.tile([1, 128], F32)
            nc.sync.dma_start(out=row[:], in_=scr[:].rearrange("p one -> one p"))
            mall = small.tile([1, 1], F32)
            nc.vector.tensor_reduce(out=mall[:], in_=row[:], axis=AX, op=ALU.max)
            nc.sync.dma_start(out=outp[:], in_=mall[:])
    nc.compile()
    return nc


def _build_main(
    general_affine: bool,
    clamp_w: bool,
    s_x_const: float,
    TOKc: int = TOK,
    K: int = INTER,
    HIDc: int = HID,
):
    TOK_T = TOKc // 128
    KT = K // 128
    KSW = 512
    KS = K // KSW
    HID_T = HIDc // 128
    XH = min(K, 2048)
    XHN = K // XH
    NB = min(512, HIDc)

    nc = bacc.Bacc("TRN2", target_bir_lowering=False, debug=False)
    x_h = nc.declare_dram_parameter("x", [TOKc, K], F32, isOutput=False)
    res_h = nc.declare_dram_parameter("res", [TOKc, HIDc], F32, isOutput=False)
    w_h = nc.declare_dram_parameter("W", [HIDc, K], F32, isOutput=False)
    scal_h = nc.declare_dram_parameter("scal", [1, 2], F32, isOutput=False)
    aff_h = nc.declare_dram_parameter("aff", [3, HIDc], F32, isOutput=False)
    out_h = nc.declare_dram_parameter("out", [TOKc, HIDc], F32, isOutput=True)
    stat_h = nc.declare_dram_parameter("stats", [1, 1], F32, isOutput=True)
    scr = nc.dram_tensor("scr", [128, 1], F32)

    with TileContext(nc) as tc, ExitStack() as ctx:
        small = ctx.enter_context(tc.tile_pool(name="small", bufs=1))
        wload = ctx.enter_context(tc.tile_pool(name="wload", bufs=2))
        wqp = ctx.enter_context(tc.tile_pool(name="wq", bufs=2))
        wqtp = ctx.enter_context(tc.tile_pool(name="wqt", bufs=1))
        xrow = ctx.enter_context(tc.tile_pool(name="xrow", bufs=3))
        xqp = ctx.enter_context(tc.tile_pool(name="xq", bufs=2))
        xqtp = ctx.enter_context(tc.tile_pool(name="xqt", bufs=4))
        resp = ctx.enter_context(tc.tile_pool(name="res", bufs=2))
        yp = ctx.enter_context(tc.tile_pool(name="y", bufs=2))
        bnp = ctx.enter_context(tc.tile_pool(name="bn", bufs=2))
        tiny = ctx.enter_context(tc.tile_pool(name="tiny", bufs=TOK_T + 2))
        psum = ctx.enter_context(tc.tile_pool(name="psum", bufs=4, space="PSUM"))

        scb = small.tile([128, 2], F32)
        nc.gpsimd.dma_start(out=scb[:], in_=scal_h[:].broadcast_to([128, 2]))
        s_w_ap = scb[:, 0:1]
        inv_ss_ap = scb[:, 1:2]

        if general_affine:
            b_rep = small.tile([128, HIDc], F32)
            g_rep = small.tile([128, HIDc], F32)
            be_rep = small.tile([128, HIDc], F32)
            nc.gpsimd.dma_start(
                out=b_rep[:], in_=aff_h[0:1, :].broadcast_to([128, HIDc]))
            nc.gpsimd.dma_start(
                out=g_rep[:], in_=aff_h[1:2, :].broadcast_to([128, HIDc]))
            nc.gpsimd.dma_start(
                out=be_rep[:], in_=aff_h[2:3, :].broadcast_to([128, HIDc]))

        # --- W stream: contiguous 128-row chunks, quantize, one transpose each
        wqT = wqtp.tile([128, KT, HIDc], BF16)
        for g in range(HID_T):
            wsl = wload.tile([128, K], F32)
            nc.sync.dma_start(out=wsl[:], in_=w_h[g * 128 : (g + 1) * 128, :])
            if clamp_w:
                nc.vector.tensor_scalar(
                    out=wsl[:], in0=wsl[:], scalar1=-CLIP, scalar2=CLIP,
                    op0=ALU.max, op1=ALU.min,
                )
            nc.vector.tensor_scalar(
                out=wsl[:], in0=wsl[:], scalar1=s_w_ap, scalar2=MAGIC,
                op0=ALU.mult, op1=ALU.add,
            )
            wq = wqp.tile([128, K], BF16)
            nc.vector.tensor_scalar(
                out=wq[:], in0=wsl[:],
                scalar1=MAGIC, scalar2=None, op0=ALU.subtract,
            )
            nc.scalar.dma_start_transpose(
                out=wqT[:, :, g * 128 : (g + 1) * 128], in_=wq[:],
            )

        # --- x rows: quantize + transpose, then matmul + residual + LN
        x0m = small.tile([128, 1], F32)
        for tt in range(TOK_T):
            xqt = xqtp.tile([128, KT, 128], BF16)
            xq = xqp.tile([128, K], BF16)
            for h in range(XHN):
                xr = xrow.tile([128, XH], F32)
                nc.sync.dma_start(
                    out=xr[:],
                    in_=x_h[tt * 128 : (tt + 1) * 128, h * XH : (h + 1) * XH],
                )
                if tt == 0 and h == 0:
                    nc.vector.tensor_reduce(
                        out=x0m[:], in_=xr[:], axis=AX, op=ALU.max,
                        apply_absolute_value=True,
                    )
                nc.vector.tensor_scalar(
                    out=xr[:], in0=xr[:], scalar1=s_x_const, scalar2=MAGIC,
                    op0=ALU.mult, op1=ALU.add,
                )
                nc.vector.tensor_scalar(
                    out=xr[:], in0=xr[:], scalar1=MAGIC, scalar2=-127.0,
                    op0=ALU.subtract, op1=ALU.max,
                )
                nc.vector.tensor_scalar(
                    out=xq[:, h * XH : (h + 1) * XH], in0=xr[:],
                    scalar1=127.0, scalar2=None, op0=ALU.min,
                )
            nc.scalar.dma_start_transpose(out=xqt[:], in_=xq[:])

            rt = resp.tile([128, HIDc], F32)
            nc.sync.dma_start(out=rt[:], in_=res_h[tt * 128 : (tt + 1) * 128, :])

            pt = psum.tile([128, HIDc], F32)
            for kt in range(KT):
                for n0 in range(0, HIDc, NB):
                    nc.tensor.matmul(
                        pt[:, n0 : n0 + NB],
                        xqt[:, kt, :],
                        wqT[:, kt, n0 : n0 + NB],
                        start=(kt == 0),
                        stop=(kt == KT - 1),
                    )

            y = yp.tile([128, HIDc], F32)
            nc.vector.tensor_scalar(
                out=y[:], in0=pt[:], scalar1=inv_ss_ap, scalar2=None, op0=ALU.mult
            )
            nc.vector.tensor_tensor(out=y[:], in0=y[:], in1=rt[:], op=ALU.add)
            if general_affine:
                nc.vector.tensor_tensor(out=y[:], in0=y[:], in1=b_rep[:], op=ALU.add)

            chunk = min(512, HIDc)
            nb = HIDc // chunk
            st6 = bnp.tile([128, nb * 6], F32)
            for i in range(nb):
                nc.vector.bn_stats(
                    out=st6[:, 6 * i : 6 * i + 6],
                    in_=y[:, i * chunk : (i + 1) * chunk],
                )
            mv = bnp.tile([128, 2], F32)
            nc.vector.bn_aggr(out=mv[:], in_=st6[:])

            t4 = tiny.tile([128, 6], F32)
            z = t4[:, 0:1]
            nc.vector.tensor_scalar(
                out=z, in0=mv[:, 1:2], scalar1=EPS, scalar2=None, op0=ALU.add
            )
            s0 = t4[:, 1:2]
            nc.scalar.activation(out=s0, in_=z, func=mybir.ActivationFunctionType.Sqrt)
            r0 = t4[:, 2:3]
            nc.vector.reciprocal(out=r0, in_=s0)
            q1 = t4[:, 3:4]
            nc.vector.tensor_tensor(out=q1, in0=r0, in1=r0, op=ALU.mult)
            nc.vector.tensor_tensor(out=q1, in0=q1, in1=z, op=ALU.mult)
            nc.vector.tensor_scalar(
                out=q1, in0=q1, scalar1=-0.5, scalar2=1.5, op0=ALU.mult, op1=ALU.add
            )
            r1 = t4[:, 4:5]
            nc.vector.tensor_tensor(out=r1, in0=r0, in1=q1, op=ALU.mult)
            nc.vector.tensor_scalar(
                out=y[:], in0=y[:], scalar1=mv[:, 0:1], scalar2=r1,
                op0=ALU.subtract, op1=ALU.mult,
            )
            if general_affine:
                nc.vector.tensor_tensor(out=y[:], in0=y[:], in1=g_rep[:], op=ALU.mult)
                nc.vector.tensor_tensor(out=y[:], in0=y[:], in1=be_rep[:], op=ALU.add)
            nc.sync.dma_start(out=out_h[tt * 128 : (tt + 1) * 128, :], in_=y[:])

        nc.sync.dma_start(out=scr[:], in_=x0m[:])
        xrowm = small.tile([1, 128], F32)
        nc.sync.dma_start(out=xrowm[:], in_=scr[:].rearrange("p one -> one p"))
        xga = small.tile([1, 1], F32)
        nc.vector.tensor_reduce(out=xga[:], in_=xrowm[:], axis=AX, op=ALU.max)
        nc.sync.dma_start(out=stat_h[:], in_=xga[:])
    nc.compile()
    return nc


def _get_nc(key, builder, *args):
    if key not in _NC_CACHE:
        _NC_CACHE[key] = builder(*args)
    return _NC_CACHE[key]


def _install_ntff_shim():
    """This image lacks ``antenv.axon_hooks``; synthesize it so
    run_bass_kernel_spmd(trace=True) can drive NTFF profiling through
    libaxon_pjrt.so's C ABI (same mechanism as trn_boot's ctypes hook)."""
    import contextlib
    import ctypes
    import sys
    import types

    if "antenv.axon_hooks" in sys.modules:
        return
    so_path = "/opt/axon/libaxon_pjrt.so"
    lib = ctypes.CDLL(so_path)
    if not hasattr(lib, "axon_start_nrt_profile"):
        return
    lib.axon_start_nrt_profile.argtypes = [
        ctypes.POINTER(ctypes.c_int64), ctypes.c_size_t,
    ]
    lib.axon_start_nrt_profile.restype = ctypes.c_int64
    lib.axon_stop_nrt_profile.argtypes = [ctypes.c_char_p]
    lib.axon_stop_nrt_profile.restype = ctypes.c_int64

    @contextlib.contextmanager
    def _hook(output_dir, device_ids):
        import jax

        jax.devices()
        if device_ids:
            ids = (ctypes.c_int64 * len(device_ids))(*device_ids)
            rc = lib.axon_start_nrt_profile(ids, len(device_ids))
        else:
            rc = lib.axon_start_nrt_profile(None, 0)
        if rc != 0:
            raise RuntimeError(f"axon_start_nrt_profile rc={rc}")
        try:
            yield
        finally:
            n = lib.axon_stop_nrt_profile(str(output_dir).encode())
            print(f"ntff profile: {n} file(s) -> {output_dir}", file=sys.stderr)

    mod = types.ModuleType("antenv.axon_hooks")
    mod.get_axon_ntff_profile_hook = lambda: _hook
    mod.set_axon_ntff_profile_hook = lambda h: None
    pkg = sys.modules.get("antenv") or types.ModuleType("antenv")
    pkg.axon_hooks = mod
    sys.modules["antenv"] = pkg
    sys.modules["antenv.axon_hooks"] = mod


def _run(nc, in_maps, label):
    import os

    trace = bool(os.environ.get("BERT_KERNEL_TRACE"))
    core_ids = list(range(len(in_maps)))
    if trace:
        try:
            _install_ntff_shim()
            r = run_bass_kernel_spmd(nc, in_maps, core_ids, trace=True)
            LAST_EXEC_NS.append((label, r.exec_time_ns))
            LAST_RESULTS[label] = r
            return r.results
        except Exception as e:  # trace plumbing must never break correctness
            print(f"trace failed ({label}): {type(e).__name__}: {e}")
    r = run_bass_kernel_spmd(nc, in_maps, core_ids, trace=False)
    return r.results


LAST_RESULTS: dict = {}


def kernel(hidden_states, input_tensor, W, b, gamma, beta):
    f32 = np.float32
    x = np.ascontiguousarray(hidden_states, dtype=f32).reshape(B * S, INTER)
    res = np.ascontiguousarray(input_tensor, dtype=f32).reshape(B * S, HID)
    Wc = np.ascontiguousarray(W, dtype=f32)
    b = np.asarray(b, f32).reshape(HID)
    gamma = np.asarray(gamma, f32).reshape(HID)
    beta = np.asarray(beta, f32).reshape(HID)

    general_affine = not (
        np.all(b == 0.0) and np.all(gamma == 1.0) and np.all(beta == 0.0)
    )
    aff = np.stack([b, gamma, beta]).astype(f32)

    # --- k1: W abs-max, one 128-row slice per core, host max-combines
    nc1 = _get_nc(("absmax", 128, INTER), _build_absmax, 128, INTER)
    slices = [Wc[i * 128 : (i + 1) * 128, :] for i in range(N_CORES)]
    r1 = _run(nc1, [{"inp": s} for s in slices], "k1_wmax")
    m_w = f32(max(f32(r["absmax"][0, 0]) for r in r1))

    m_eff = min(m_w, f32(CLIP))
    s_w = f32(127.0) / f32(m_eff)
    inv_ss = (f32(m_eff) / f32(127.0)) * (f32(CLIP) / f32(127.0))
    clamp_w = bool(m_w > CLIP)
    s_x = float(f32(127.0) / f32(CLIP))

    def run_main(s_x_const, inv_ss_val):
        nc2 = _get_nc(
            ("main", general_affine, clamp_w, float(s_x_const)),
            _build_main, general_affine, clamp_w, float(s_x_const),
        )
        scal = np.array([[s_w, inv_ss_val]], f32)
        in_maps = [
            {
                "x": x[i * TOK : (i + 1) * TOK],
                "res": res[i * TOK : (i + 1) * TOK],
                "W": Wc,
                "scal": scal,
                "aff": aff,
            }
            for i in range(N_CORES)
        ]
        r2 = _run(nc2, in_maps, "k2_main")
        out = np.concatenate([r["out"] for r in r2], axis=0)
        xmax = max(float(r["stats"][0, 0]) for r in r2)
        return out, xmax

    out, xmax = run_main(s_x, inv_ss)
    if xmax < CLIP:
        # clip never saturated in the sampled tiles: prove/refute s_x=127/2.5
        # with a full device abs-max over x, and recompute if refuted.
        ncx = _get_nc(("absmax", TOK, INTER), _build_absmax, TOK, INTER)
        rx = _run(ncx, [{"inp": x[i * TOK : (i + 1) * TOK]} for i in range(N_CORES)],
                  "kx_xmax")
        gmax = f32(max(f32(r["absmax"][0, 0]) for r in rx))
        if gmax < CLIP:
            m_x = f32(min(gmax, f32(CLIP)))
            s_x2 = f32(127.0) / m_x
            inv2 = (f32(m_x) / f32(127.0)) * (f32(m_eff) / f32(127.0))
            out, _ = run_main(float(s_x2), inv2)

    return out.reshape(B, S, HID).astype(np.float32)


# revision 9
# speedup vs baseline: 1.2440x; 1.0856x over previous
"""Trainium2 Bass kernel for quantized BertOutput (BiT SymQuantizer 8-bit
linear + residual + LayerNorm), data-parallel over 8 NeuronCores.

Contract: kernel(**inputs) takes the FULL inputs from setup_inputs() and
returns the FULL [4, 4096, 1024] fp32 output.

Math per core (token shard of 2048 rows):
  k_x = clip(round_half_even(x * s_x), -127, 127)   (integers, bf16-exact)
  k_w = round_half_even(w * s_w)
  h   = (k_x @ k_w.T) * inv_ss                       (bf16 matmul, fp32 PSUM)
  y   = h + res ; out = (y - mean(y)) * rsqrt(var(y) + eps)

The BiT layerwise scales need global abs-maxes *before* any quantized tile
can be produced, so they are obtained in a tiny first launch (k1): each core
reduces one 128-row slice of W on-device; the host only max-combines the 8
device-computed scalars and forms s_w = 127/min(m,2.5) in fp32 (bit-identical
to the reference's divide).  s_x = 127/2.5 is *assumed* (the clip saturates
for any realistic input) and *proven* on device: k2 reports an abs-max over
its first row-tile, and if no core saw an element >= 2.5 the host falls back
to a general-scale kernel, so the result is correct for arbitrary inputs.
"""

from contextlib import ExitStack

import numpy as np

import concourse.bacc as bacc
import concourse.bass as bass
import concourse.mybir as mybir
from concourse import bass_isa, masks  # noqa: F401
from concourse.bass_utils import run_bass_kernel_spmd
from concourse.tile import TileContext

F32 = mybir.dt.float32
BF16 = mybir.dt.bfloat16
MAGIC = float(np.float32(12582912.0))  # 1.5 * 2**23 -> fp32 RNE round trick
AX = mybir.AxisListType.X
ALU = mybir.AluOpType

B, S, INTER, HID = 4, 4096, 4096, 1024
N_CORES = 8
TOK = (B * S) // N_CORES  # 2048 tokens per core
CLIP = 2.5
EPS = 1e-12

_NC_CACHE: dict = {}
LAST_EXEC_NS: list = []  # (label, exec_time_ns) when BERT_KERNEL_TRACE=1


def _build_absmax(rows: int, cols: int):
    """out[0,0] = max(|inp|) over [rows, cols]."""
    nc = bacc.Bacc("TRN2", target_bir_lowering=False, debug=False)
    inp = nc.declare_dram_parameter("inp", [rows, cols], F32, isOutput=False)
    outp = nc.declare_dram_parameter("absmax", [1, 1], F32, isOutput=True)
    scr = nc.dram_tensor("scr", [128, 1], F32)
    n_chunks = rows // 128
    with TileContext(nc) as tc:
        with tc.tile_pool(name="pool", bufs=2) as pool, tc.tile_pool(
            name="small", bufs=1
        ) as small:
            cols_t = small.tile([128, max(n_chunks, 2)], F32)
            for c in range(n_chunks):
                t = pool.tile([128, cols], F32)
                nc.sync.dma_start(out=t[:], in_=inp[c * 128 : (c + 1) * 128, :])
                nc.vector.tensor_reduce(
                    out=cols_t[:, c : c + 1], in_=t[:], axis=AX,
                    op=ALU.max, apply_absolute_value=True,
                )
            m = small.tile([128, 1], F32)
            nc.vector.tensor_reduce(
                out=m[:], in_=cols_t[:, 0:n_chunks], axis=AX, op=ALU.max
            )
            nc.sync.dma_start(out=scr[:], in_=m[:])
            row = small.tile([1, 128], F32)
            nc.sync.dma_start(out=row[:], in_=scr[:].rearrange("p one -> one p"))
            mall = small.tile([1, 1], F32)
            nc.vector.tensor_reduce(out=mall[:], in_=row[:], axis=AX, op=ALU.max)
            nc.sync.dma_start(out=outp[:], in_=mall[:])
    nc.compile()
    return nc


def _build_main(
    general_affine: bool,
    clamp_w: bool,
    s_x_const: float,
    TOKc: int = TOK,
    K: int = INTER,
    HIDc: int = HID,
):
    TOK_T = TOKc // 128
    KT = K // 128
    KSW = 512
    KS = K // KSW
    HID_T = HIDc // 128
    XH = min(K, 2048)
    XHN = K // XH
    NB = min(512, HIDc)

    nc = bacc.Bacc("TRN2", target_bir_lowering=False, debug=False)
    x_h = nc.declare_dram_parameter("x", [TOKc, K], F32, isOutput=False)
    res_h = nc.declare_dram_parameter("res", [TOKc, HIDc], F32, isOutput=False)
    w_h = nc.declare_dram_parameter("W", [HIDc, K], F32, isOutput=False)
    scal_h = nc.declare_dram_parameter("scal", [1, 2], F32, isOutput=False)
    aff_h = nc.declare_dram_parameter("aff", [3, HIDc], F32, isOutput=False)
    out_h = nc.declare_dram_parameter("out", [TOKc, HIDc], F32, isOutput=True)
    stat_h = nc.declare_dram_parameter("stats", [1, 1], F32, isOutput=True)
    scr = nc.dram_tensor("scr", [128, 1], F32)

    with TileContext(nc) as tc, ExitStack() as ctx:
        small = ctx.enter_context(tc.tile_pool(name="small", bufs=1))
        wload = ctx.enter_context(tc.tile_pool(name="wload", bufs=2))
        wqp = ctx.enter_context(tc.tile_pool(name="wq", bufs=2))
        wqtp = ctx.enter_context(tc.tile_pool(name="wqt", bufs=1))
        xrow = ctx.enter_context(tc.tile_pool(name="xrow", bufs=3))
        xqp = ctx.enter_context(tc.tile_pool(name="xq", bufs=2))
        xqtp = ctx.enter_context(tc.tile_pool(name="xqt", bufs=4))
        resp = ctx.enter_context(tc.tile_pool(name="res", bufs=2))
        yp = ctx.enter_context(tc.tile_pool(name="y", bufs=2))
        bnp = ctx.enter_context(tc.tile_pool(name="bn", bufs=2))
        tiny = ctx.enter_context(tc.tile_pool(name="tiny", bufs=TOK_T + 2))
        psum = ctx.enter_context(tc.tile_pool(name="psum", bufs=3, space="PSUM"))
        psumt = ctx.enter_context(tc.tile_pool(name="psumt", bufs=2, space="PSUM"))

        scb = small.tile([128, 2], F32)
        nc.gpsimd.dma_start(out=scb[:], in_=scal_h[:].broadcast_to([128, 2]))
        s_w_ap = scb[:, 0:1]
        inv_ss_ap = scb[:, 1:2]

        if general_affine:
            b_rep = small.tile([128, HIDc], F32)
            g_rep = small.tile([128, HIDc], F32)
            be_rep = small.tile([128, HIDc], F32)
            nc.gpsimd.dma_start(
                out=b_rep[:], in_=aff_h[0:1, :].broadcast_to([128, HIDc]))
            nc.gpsimd.dma_start(
                out=g_rep[:], in_=aff_h[1:2, :].broadcast_to([128, HIDc]))
            nc.gpsimd.dma_start(
                out=be_rep[:], in_=aff_h[2:3, :].broadcast_to([128, HIDc]))

        # --- W stream: contiguous 128-row chunks, quantize, PE-transpose into
        # k-split wqT tiles (parallel writers; matmul kt gates on one k-group)
        ident = small.tile([128, 128], BF16)
        masks.make_identity(nc, ident[:])
        KG = 4                      # kt per k-group (512 k per group)
        NKG = KT // KG
        wqts = [
            wqtp.tile([128, KG, HIDc], BF16, name=f"wqt{s}", tag=f"wqt{s}")
            for s in range(NKG)
        ]
        for g in range(HID_T):
            wsl = wload.tile([128, K], F32)
            nc.sync.dma_start(out=wsl[:], in_=w_h[g * 128 : (g + 1) * 128, :])
            if clamp_w:
                nc.vector.tensor_scalar(
                    out=wsl[:], in0=wsl[:], scalar1=-CLIP, scalar2=CLIP,
                    op0=ALU.max, op1=ALU.min,
                )
            nc.vector.tensor_scalar(
                out=wsl[:], in0=wsl[:], scalar1=s_w_ap, scalar2=MAGIC,
                op0=ALU.mult, op1=ALU.add,
            )
            wq = wqp.tile([128, K], BF16)
            nc.vector.tensor_scalar(
                out=wq[:], in0=wsl[:],
                scalar1=MAGIC, scalar2=None, op0=ALU.subtract,
            )
            for s in range(NKG):
                pst = psumt.tile([128, KG * 128], BF16)
                for j in range(KG):
                    kt = s * KG + j
                    nc.tensor.transpose(
                        pst[:, j * 128 : (j + 1) * 128],
                        wq[:, kt * 128 : (kt + 1) * 128],
                        ident[:],
                    )
                nc.scalar.copy(
                    out=wqts[s][:, :, g * 128 : (g + 1) * 128], in_=pst[:],
                )

        # --- x rows: quantize + transpose, then matmul + residual + LN
        x0m = small.tile([128, 1], F32)
        for tt in range(TOK_T):
            xqt = xqtp.tile([128, KT, 128], BF16)
            xq = xqp.tile([128, K], BF16)
            for h in range(XHN):
                xr = xrow.tile([128, XH], F32)
                nc.gpsimd.dma_start(
                    out=xr[:],
                    in_=x_h[tt * 128 : (tt + 1) * 128, h * XH : (h + 1) * XH],
                )
                if tt == 0 and h == 0:
                    nc.vector.tensor_reduce(
                        out=x0m[:], in_=xr[:], axis=AX, op=ALU.max,
                        apply_absolute_value=True,
                    )
                nc.vector.tensor_scalar(
                    out=xr[:], in0=xr[:], scalar1=s_x_const, scalar2=MAGIC,
                    op0=ALU.mult, op1=ALU.add,
                )
                nc.vector.tensor_scalar(
                    out=xr[:], in0=xr[:], scalar1=MAGIC, scalar2=-127.0,
                    op0=ALU.subtract, op1=ALU.max,
                )
                nc.vector.tensor_scalar(
                    out=xq[:, h * XH : (h + 1) * XH], in0=xr[:],
                    scalar1=127.0, scalar2=None, op0=ALU.min,
                )
            nc.scalar.dma_start_transpose(out=xqt[:], in_=xq[:])

            rt = resp.tile([128, HIDc], F32)
            nc.gpsimd.dma_start(out=rt[:], in_=res_h[tt * 128 : (tt + 1) * 128, :])

            pt = psum.tile([128, HIDc], F32)
            for kt in range(KT):
                for n0 in range(0, HIDc, NB):
                    nc.tensor.matmul(
                        pt[:, n0 : n0 + NB],
                        xqt[:, kt, :],
                        wqts[kt // KG][:, kt % KG, n0 : n0 + NB],
                        start=(kt == 0),
                        stop=(kt == KT - 1),
                    )

            y = yp.tile([128, HIDc], F32)
            nc.vector.tensor_scalar(
                out=y[:], in0=pt[:], scalar1=inv_ss_ap, scalar2=None, op0=ALU.mult
            )
            nc.vector.tensor_tensor(out=y[:], in0=y[:], in1=rt[:], op=ALU.add)
            if general_affine:
                nc.vector.tensor_tensor(out=y[:], in0=y[:], in1=b_rep[:], op=ALU.add)

            chunk = min(512, HIDc)
            nb = HIDc // chunk
            st6 = bnp.tile([128, nb * 6], F32)
            for i in range(nb):
                nc.vector.bn_stats(
                    out=st6[:, 6 * i : 6 * i + 6],
                    in_=y[:, i * chunk : (i + 1) * chunk],
                )
            mv = bnp.tile([128, 2], F32)
            nc.vector.bn_aggr(out=mv[:], in_=st6[:])

            t4 = tiny.tile([128, 6], F32)
            z = t4[:, 0:1]
            nc.vector.tensor_scalar(
                out=z, in0=mv[:, 1:2], scalar1=EPS, scalar2=None, op0=ALU.add
            )
            s0 = t4[:, 1:2]
            nc.scalar.activation(out=s0, in_=z, func=mybir.ActivationFunctionType.Sqrt)
            r0 = t4[:, 2:3]
            nc.vector.reciprocal(out=r0, in_=s0)
            q1 = t4[:, 3:4]
            nc.vector.tensor_tensor(out=q1, in0=r0, in1=r0, op=ALU.mult)
            nc.vector.tensor_tensor(out=q1, in0=q1, in1=z, op=ALU.mult)
            nc.vector.tensor_scalar(
                out=q1, in0=q1, scalar1=-0.5, scalar2=1.5, op0=ALU.mult, op1=ALU.add
            )
            r1 = t4[:, 4:5]
            nc.vector.tensor_tensor(out=r1, in0=r0, in1=q1, op=ALU.mult)
            nc.vector.tensor_scalar(
                out=y[:], in0=y[:], scalar1=mv[:, 0:1], scalar2=r1,
                op0=ALU.subtract, op1=ALU.mult,
            )
            if general_affine:
                nc.vector.tensor_tensor(out=y[:], in0=y[:], in1=g_rep[:], op=ALU.mult)
                nc.vector.tensor_tensor(out=y[:], in0=y[:], in1=be_rep[:], op=ALU.add)
            nc.gpsimd.dma_start(out=out_h[tt * 128 : (tt + 1) * 128, :], in_=y[:])

        nc.sync.dma_start(out=scr[:], in_=x0m[:])
        xrowm = small.tile([1, 128], F32)
        nc.sync.dma_start(out=xrowm[:], in_=scr[:].rearrange("p one -> one p"))
        xga = small.tile([1, 1], F32)
        nc.vector.tensor_reduce(out=xga[:], in_=xrowm[:], axis=AX, op=ALU.max)
        nc.sync.dma_start(out=stat_h[:], in_=xga[:])
    nc.compile()
    return nc


def _get_nc(key, builder, *args):
    if key not in _NC_CACHE:
        _NC_CACHE[key] = builder(*args)
    return _NC_CACHE[key]


def _install_ntff_shim():
    """This image lacks ``antenv.axon_hooks``; synthesize it so
    run_bass_kernel_spmd(trace=True) can drive NTFF profiling through
    libaxon_pjrt.so's C ABI (same mechanism as trn_boot's ctypes hook)."""
    import contextlib
    import ctypes
    import sys
    import types

    if "antenv.axon_hooks" in sys.modules:
        return
    so_path = "/opt/axon/libaxon_pjrt.so"
    lib = ctypes.CDLL(so_path)
    if not hasattr(lib, "axon_start_nrt_profile"):
        return
    lib.axon_start_nrt_profile.argtypes = [
        ctypes.POINTER(ctypes.c_int64), ctypes.c_size_t,
    ]
    lib.axon_start_nrt_profile.restype = ctypes.c_int64
    lib.axon_stop_nrt_profile.argtypes = [ctypes.c_char_p]
    lib.axon_stop_nrt_profile.restype = ctypes.c_int64

    @contextlib.contextmanager
    def _hook(output_dir, device_ids):
        import jax

        jax.devices()
        if device_ids:
            ids = (ctypes.c_int64 * len(device_ids))(*device_ids)
            rc = lib.axon_start_nrt_profile(ids, len(device_ids))
        else:
            rc = lib.axon_start_nrt_profile(None, 0)
        if rc != 0:
            raise RuntimeError(f"axon_start_nrt_profile rc={rc}")
        try:
            yield
        finally:
            n = lib.axon_stop_nrt_profile(str(output_dir).encode())
            print(f"ntff profile: {n} file(s) -> {output_dir}", file=sys.stderr)

    mod = types.ModuleType("antenv.axon_hooks")
    mod.get_axon_ntff_profile_hook = lambda: _hook
    mod.set_axon_ntff_profile_hook = lambda h: None
    pkg = sys.modules.get("antenv") or types.ModuleType("antenv")
    pkg.axon_hooks = mod
    sys.modules["antenv"] = pkg
    sys.modules["antenv.axon_hooks"] = mod


def _run(nc, in_maps, label):
    import os

    trace = bool(os.environ.get("BERT_KERNEL_TRACE"))
    core_ids = list(range(len(in_maps)))
    if trace:
        try:
            _install_ntff_shim()
            r = run_bass_kernel_spmd(nc, in_maps, core_ids, trace=True)
            LAST_EXEC_NS.append((label, r.exec_time_ns))
            LAST_RESULTS[label] = r
            return r.results
        except Exception as e:  # trace plumbing must never break correctness
            print(f"trace failed ({label}): {type(e).__name__}: {e}")
    r = run_bass_kernel_spmd(nc, in_maps, core_ids, trace=False)
    return r.results


LAST_RESULTS: dict = {}


def kernel(hidden_states, input_tensor, W, b, gamma, beta):
    f32 = np.float32
    x = np.ascontiguousarray(hidden_states, dtype=f32).reshape(B * S, INTER)
    res = np.ascontiguousarray(input_tensor, dtype=f32).reshape(B * S, HID)
    Wc = np.ascontiguousarray(W, dtype=f32)
    b = np.asarray(b, f32).reshape(HID)
    gamma = np.asarray(gamma, f32).reshape(HID)
    beta = np.asarray(beta, f32).reshape(HID)

    general_affine = not (
        np.all(b == 0.0) and np.all(gamma == 1.0) and np.all(beta == 0.0)
    )
    aff = np.stack([b, gamma, beta]).astype(f32)

    # --- k1: W abs-max, one 128-row slice per core, host max-combines
    nc1 = _get_nc(("absmax", 128, INTER), _build_absmax, 128, INTER)
    slices = [Wc[i * 128 : (i + 1) * 128, :] for i in range(N_CORES)]
    r1 = _run(nc1, [{"inp": s} for s in slices], "k1_wmax")
    m_w = f32(max(f32(r["absmax"][0, 0]) for r in r1))

    m_eff = min(m_w, f32(CLIP))
    s_w = f32(127.0) / f32(m_eff)
    inv_ss = (f32(m_eff) / f32(127.0)) * (f32(CLIP) / f32(127.0))
    clamp_w = bool(m_w > CLIP)
    s_x = float(f32(127.0) / f32(CLIP))

    def run_main(s_x_const, inv_ss_val):
        nc2 = _get_nc(
            ("main", general_affine, clamp_w, float(s_x_const)),
            _build_main, general_affine, clamp_w, float(s_x_const),
        )
        scal = np.array([[s_w, inv_ss_val]], f32)
        in_maps = [
            {
                "x": x[i * TOK : (i + 1) * TOK],
                "res": res[i * TOK : (i + 1) * TOK],
                "W": Wc,
                "scal": scal,
                "aff": aff,
            }
            for i in range(N_CORES)
        ]
        r2 = _run(nc2, in_maps, "k2_main")
        out = np.concatenate([r["out"] for r in r2], axis=0)
        xmax = max(float(r["stats"][0, 0]) for r in r2)
        return out, xmax

    out, xmax = run_main(s_x, inv_ss)
    if xmax < CLIP:
        # clip never saturated in the sampled tiles: prove/refute s_x=127/2.5
        # with a full device abs-max over x, and recompute if refuted.
        ncx = _get_nc(("absmax", TOK, INTER), _build_absmax, TOK, INTER)
        rx = _run(ncx, [{"inp": x[i * TOK : (i + 1) * TOK]} for i in range(N_CORES)],
                  "kx_xmax")
        gmax = f32(max(f32(r["absmax"][0, 0]) for r in rx))
        if gmax < CLIP:
            m_x = f32(min(gmax, f32(CLIP)))
            s_x2 = f32(127.0) / m_x
            inv2 = (f32(m_x) / f32(127.0)) * (f32(m_eff) / f32(127.0))
            out, _ = run_main(float(s_x2), inv2)

    return out.reshape(B, S, HID).astype(np.float32)
